# revision 41
# baseline (speedup 1.0000x reference)
"""Converse2D-Up (FFT deconvolution upsampler) as a Bass/Tile kernel for TRN2.

Math (validated against the jax reference; bf16 device pipeline sims to
rel-l2 ~5.4e-3 vs the harness gate of 2e-2):

The whole pipeline before the final gelu is linear in x and channel-wise.
With xp = wrap-pad(x) (132x132), Y = FFT132(xp) = G @ x @ G^T where
G = F132 @ P (132x128, P = periodic pad selection).  The reference's
264-point spectral transfer function H (built from weight/bias only) is
Hermitian, so out = crop(real(IFFT264(H . tile(Y)))) decomposes into 4
polyphase outputs out_dd = real(IFFT132(Kdd_hat . Y)) with per-channel
precomputed spectra Kdd_hat; the crop leaves exactly 128 rows/cols per
phase.  Hermitian symmetry further means only columns v=0..66 of
Kdd_hat.Y are needed.

Device pipeline per image (all matmuls bf16 moving+stationary, fp32 PSUM):
  A : pA[m,u']    = x^T @ gt                 1 mm  N=264
  B : pY[u,(r|i)] = r1-slices @ [Ma|Mb]      2 mm  N=134  (hi rows 0..127)
      pYlo4 (4 lo rows, 4 imgs at part 32b)  2 mm  N=134  per image
  cmul (DVE, bf16 2x): FX = Kdd .* Y         3 ops/img hi + 3 ops/ch lo
  C': pT1[v,(T1r|T1i)] per phase p:
        fxr_hi_p @ [C|S] + fxi_hi_p @ [-S|C] (+ K=4 lo terms)
                                             16 mm N=256
  D : pD[x, p*128+y] = T1r_p @ RC + T1i_p @ RS   8 mm N=128
  gelu + phase interleave fused in ScalarE eviction, 1 DMA per image.

Sharding: 8 channels per core x 4 batch images; weight/bias-derived
spectra are host-precomputed fp64 constants cast to bf16.
"""

import os

import numpy as np
import ml_dtypes

import concourse.bass as bass
import concourse.mybir as mybir
import concourse.tile as tile
from concourse import bacc
from concourse.bass import ts
from concourse.bass_utils import run_bass_kernel_spmd

F32 = mybir.dt.float32
BF16 = mybir.dt.bfloat16
AF = mybir.ActivationFunctionType
BF16NP = ml_dtypes.bfloat16

SCALE = 2
PAD = 2
EPS = 1e-5
N0 = 128           # input spatial size
NP = N0 + 2 * PAD  # 132 padded
NU = NP * SCALE    # 264 upsampled
NV = NP // 2 + 1   # 67 unique spectral columns
B = 4
C = 64
NCORES = 8
CPC = C // NCORES  # 8 channels per core
NIMG = B * CPC     # 32 images per core

LAST_EXEC_NS = None  # set by kernel() when tracing is enabled


# --------------------------------------------------------------------------
# host-side constant precompute (weight/bias -> per-channel spectra)
# --------------------------------------------------------------------------

def _host_constants(weight, bias):
    w64 = np.asarray(weight, dtype=np.float64)
    b64 = np.asarray(bias, dtype=np.float64)

    # FB = p2o(weight): 264-point OTF of the rolled 3x3 PSF, per channel
    k_h, k_w = w64.shape[-2:]
    otf = np.zeros((C, NU, NU), dtype=np.complex128)
    otf[:, :k_h, :k_w] = w64[0]
    otf = np.roll(otf, (-(k_h // 2), -(k_w // 2)), axis=(-2, -1))
    FB = np.fft.fftn(otf, axes=(-2, -1))                      # (C,264,264)

    biaseps = 1.0 / (1.0 + np.exp(-(b64.reshape(C) - 9.0))) + EPS  # (C,)
    be = biaseps[:, None, None]

    u = np.arange(NU)
    Dr = 1 + np.exp(-2j * np.pi * u / NU)
    D = Dr[:, None] * Dr[None, :]                             # (264,264)

    Gh = np.conj(FB) + be * D[None]
    FBG = FB * Gh

    def quadmean(A):
        return 0.25 * (A[:, :NP, :NP] + A[:, NP:, :NP]
                       + A[:, :NP, NP:] + A[:, NP:, NP:])

    M1 = quadmean(FBG)
    invW = quadmean(np.abs(FB) ** 2)
    M2 = M1 / (invW + be)
    H = (Gh - np.conj(FB) * np.tile(M2, (1, SCALE, SCALE))) / be   # (C,264,264)

    hr = np.fft.ifft2(H, axes=(-2, -1)).real                  # H Hermitian
    # polyphase spectra: Kdd_hat[c,dx,dy] = FFT132(hr[c, dx::2, dy::2])
    kdd = np.empty((C, 4, NP, NV), dtype=np.complex128)
    for dx in range(2):
        for dy in range(2):
            kh = np.fft.fft2(hr[:, dx::2, dy::2], axes=(-2, -1))
            kdd[:, dx * 2 + dy] = kh[:, :, :NV]

    # cmul packing: k[c, u, g, p, h, v] = (g==0 ? Kr : Ki)[c, p, u, v]
    # (h duplicates the 67-wide block so one DVE mul covers Yr and Yi)
    kr = kdd.real.astype(np.float32)                  # (C, 4, 132, 67)
    ki = kdd.imag.astype(np.float32)
    kg = np.stack([kr, ki], axis=1)                   # (C, 2, 4, 132, 67)
    kg = np.repeat(kg[:, :, :, :, None, :], 2, axis=4)  # (C,2,4,132,2,67)
    kpack = kg.transpose(0, 3, 1, 2, 4, 5).reshape(C, NP, 2 * 4 * 2 * NV)
    # rows 0..127 = hi; lo rows u=128..131 replicated at offsets 0, 32
    # (two images of a pair share one lo tile; AP base partitions are
    # restricted to {0,32,64})
    k_hi = kpack[:, :128]                             # (C, 128, 1072)
    k_lo = kpack[:, 128:]                             # (C, 4, 1072)
    k_lo2 = np.zeros((C, 36, 2 * 4 * 2 * NV), dtype=np.float32)
    for b_ in range(2):
        k_lo2[:, 32 * b_:32 * b_ + 4] = k_lo

    # forward matrix G = F132 @ P  (132x128 complex)
    P = np.zeros((NP, N0))
    for m in range(NP):
        P[m, (m - PAD) % N0] = 1.0
    F132 = np.exp(-2j * np.pi * np.outer(np.arange(NP), np.arange(NP)) / NP)
    G = F132 @ P
    GrT, GiT = G.real.T, G.imag.T                              # (128,132)

    gt = np.concatenate([GrT, GiT], axis=1)                    # (128,264)
    mab = np.concatenate(
        [GrT[:, :NV], GiT[:, :NV], -GiT[:, :NV], GrT[:, :NV]], axis=1
    )                                                          # (128,268)

    # inverse matrix, rows i in [2,130) of iF132/132
    Ai = np.exp(2j * np.pi * np.outer(np.arange(2, 130), np.arange(NP)) / NP) / NP
    CT, ST = Ai.real.T, Ai.imag.T                              # (132,128)
    n12 = np.concatenate([CT[:128], ST[:128], -ST[:128], CT[:128]], axis=1)
    # lo N-matrix for the K=8-stacked lo matmul ([fxr(4); fxi(4)] rows):
    # rows 0..3 = [C_lo | S_lo], rows 4..7 = [-S_lo | C_lo]; replicated
    # at partition offsets 0, 32 (one image pair per tile)
    n12lo = np.zeros((40, 256), dtype=np.float64)
    lo8 = np.concatenate([
        np.concatenate([CT[128:], ST[128:]], axis=1),
        np.concatenate([-ST[128:], CT[128:]], axis=1)], axis=0)  # (8, 256)
    for b_ in range(2):
        n12lo[32 * b_:32 * b_ + 8] = lo8

    w_v = np.ones(NV)
    w_v[1:NV - 1] = 2.0
    RC = (Ai.real[:, :NV] * w_v[None, :]).T                    # (67,128)
    RS = (-Ai.imag[:, :NV] * w_v[None, :]).T
    rcs = np.concatenate([RC, RS], axis=1)                     # (67,256)

    def b16(a):
        return np.ascontiguousarray(np.asarray(a, dtype=np.float32)).astype(BF16NP)

    return {
        "k_hi": b16(k_hi),
        "k_lo2": b16(k_lo2),
        "gt": b16(gt),
        "mab": b16(mab),
        "n12": b16(n12),
        "n12lo": b16(n12lo),
        "rcs": b16(rcs),
    }


# --------------------------------------------------------------------------
# device kernel
# --------------------------------------------------------------------------

def build_nc(n_chan=CPC, n_batch=B, gelu=True):
    act_fn = AF.Gelu if gelu else AF.Copy
    n_img = n_chan * n_batch
    nc = bacc.Bacc("TRN2", target_bir_lowering=False, debug=False,
                   enable_asserts=False)

    # x pre-transposed host-side to [chan, row, batch, col] so the input
    # DMA is contiguous per partition (2KB packets instead of 512B)
    x_t = nc.dram_tensor("x", [n_chan, N0, n_batch, N0], F32,
                         kind="ExternalInput")
    khi_t = nc.dram_tensor("k_hi", [n_chan, 128, 16 * NV], BF16,
                           kind="ExternalInput")
    klo_t = nc.dram_tensor("k_lo2", [n_chan, 36, 16 * NV], BF16,
                           kind="ExternalInput")
    gt_t = nc.dram_tensor("gt", [128, 2 * NP], BF16, kind="ExternalInput")
    mab_t = nc.dram_tensor("mab", [128, 4 * NV], BF16, kind="ExternalInput")
    n12_t = nc.dram_tensor("n12", [128, 512], BF16, kind="ExternalInput")
    n12lo_t = nc.dram_tensor("n12lo", [40, 256], BF16, kind="ExternalInput")
    rcs_t = nc.dram_tensor("rcs", [NV, 256], BF16, kind="ExternalInput")
    out_t = nc.dram_tensor("out", [n_img, 2 * N0, 2 * N0], F32,
                           kind="ExternalOutput")

    PH16 = 16 * NV        # 1072, cmul packed width
    PH4 = 4 * NV          # 268
    with tile.TileContext(nc) as tc:
        with (
            tc.tile_pool(name="consts", bufs=1) as cpool,
            tc.tile_pool(name="kdd", bufs=2) as kpool,
            tc.tile_pool(name="xin", bufs=2) as xpool,
            tc.tile_pool(name="r1", bufs=2) as r1pool,
            tc.tile_pool(name="ysb", bufs=4) as ypool,
            tc.tile_pool(name="ylo", bufs=2) as ylopool,
            tc.tile_pool(name="prod", bufs=2) as prodpool,
            tc.tile_pool(name="fx", bufs=2) as fxpool,
            tc.tile_pool(name="fxlo", bufs=2) as fxlopool,
            tc.tile_pool(name="t1", bufs=2) as t1pool,
            tc.tile_pool(name="osb", bufs=2) as opool,
            tc.tile_pool(name="ppa", bufs=1, space="PSUM") as ppa_pool,
            tc.tile_pool(name="ppy", bufs=1, space="PSUM") as ppy_pool,
            tc.tile_pool(name="pt1", bufs=2, space="PSUM") as pt1_pool,
            tc.tile_pool(name="ppd", bufs=2, space="PSUM") as ppd_pool,
        ):
            gt = cpool.tile([128, 2 * NP], BF16)
            nc.sync.dma_start(gt[:], gt_t[:])
            mab = cpool.tile([128, PH4], BF16)
            nc.sync.dma_start(mab[:], mab_t[:])
            n12 = cpool.tile([128, 512], BF16)
            nc.sync.dma_start(n12[:], n12_t[:])
            n12lo = cpool.tile([40, 256], BF16)
            nc.sync.dma_start(n12lo[:], n12lo_t[:])
            rcs = cpool.tile([NV, 256], BF16)
            nc.sync.dma_start(rcs[:], rcs_t[:])

            # deferred stage-D state from the previous image
            pending = []

            def emit_D(st):
                (t1sb, img) = st
                pD = ppd_pool.tile([128, 512], F32, tag="pD")
                for p in range(4):
                    o = pD[:, ts(p, 128)]
                    nc.tensor.matmul(o, t1sb[:, p * 256:p * 256 + 128],
                                     rcs[:, 0:128], start=True, stop=False)
                    nc.tensor.matmul(o, t1sb[:, p * 256 + 128:p * 256 + 256],
                                     rcs[:, 128:256], start=False, stop=True)
                # gelu + phase interleave: otile[x, (dx, 2y+dy)]
                otile = opool.tile([128, 512], F32, tag="otile")
                for dx in range(2):
                    nc.scalar.activation(
                        otile[:, ts(dx, 256)].rearrange("p (v d) -> p d v", d=2),
                        pD[:, ts(dx, 256)].rearrange("p (d v) -> p d v", d=2),
                        act_fn)
                eng = nc.gpsimd if img % 2 == 0 else nc.sync
                eng.dma_start(
                    out_t[img].rearrange("(x d) y -> x d y", d=2),
                    otile[:].rearrange("p (d y) -> p d y", d=2))

            def prep_channel(ci):
                # x first (it gates the first matmul), split in halves so
                # stage A of image 0 starts before the whole channel lands;
                # bulky K spectra go on the idle gpsimd DMA queue
                xin = xpool.tile([N0, B * N0], F32, tag="xin")
                xb = xpool.tile([N0, B * N0], BF16, tag="xb")
                for h in range(2):
                    sl = slice(h * (B // 2) * N0, (h + 1) * (B // 2) * N0)
                    nc.sync.dma_start(
                        xin[:, sl].rearrange("p (b c) -> p b c", b=B // 2),
                        x_t[ci, :, h * 2:(h + 1) * 2, :])
                    nc.scalar.activation(xb[:, sl], xin[:, sl], AF.Copy)
                k_hi = kpool.tile([128, PH16], BF16, tag="k_hi")
                nc.gpsimd.dma_start(k_hi[:], khi_t[ci])
                k_lo2 = kpool.tile([36, PH16], BF16, tag="k_lo2")
                nc.gpsimd.dma_start(k_lo2[:], klo_t[ci])
                return k_hi, k_lo2, xb

            nxt = prep_channel(0)
            for ci in range(n_chan):
                k_hi, k_lo2, xb = nxt

                for pr in range(n_batch // 2):
                    # ---- stages A+B for the two images of this pair ----
                    # pYpair: [img0 Y-hi | img1 Y-hi | Y-lo (both, at
                    # partition offsets 0/32)] in one PSUM bank
                    r1ch = r1pool.tile([128, 2 * 2 * NP], BF16, tag="r1ch")
                    pYpair = ppy_pool.tile([128, 6 * NV], F32, tag="pYpair")
                    yhis = []
                    for sub in range(2):
                        bi = 2 * pr + sub
                        pA = ppa_pool.tile([128, 2 * NP], F32, tag="pA")
                        nc.tensor.matmul(pA[:], xb[:, ts(bi, N0)], gt[:],
                                         start=True, stop=True)
                        r1 = r1ch[:, sub * 2 * NP:(sub + 1) * 2 * NP]
                        nc.scalar.activation(r1, pA[:], AF.Copy)

                        pY = pYpair[:, sub * 2 * NV:(sub + 1) * 2 * NV]
                        nc.tensor.matmul(pY, r1[:, 0:128], mab[:, 0:2 * NV],
                                         start=True, stop=False)
                        nc.tensor.matmul(pY, r1[:, NP:NP + 128],
                                         mab[:, 2 * NV:4 * NV],
                                         start=False, stop=True)
                        plo = pYpair[32 * sub:32 * sub + 4, 4 * NV:6 * NV]
                        nc.tensor.matmul(plo, r1[:, 128:NP], mab[:, 0:2 * NV],
                                         start=True, stop=False)
                        nc.tensor.matmul(plo, r1[:, NP + 128:2 * NP],
                                         mab[:, 2 * NV:4 * NV],
                                         start=False, stop=True)
                        y_hi = ypool.tile([128, 2 * NV], BF16, tag="y_hi")
                        nc.scalar.activation(y_hi[:], pY, AF.Copy)
                        yhis.append(y_hi)

                    # ---- lo cmul, once per pair (DVE, bf16 2x) ----
                    ylo2 = ylopool.tile([36, 2 * NV], BF16, tag="ylo2")
                    nc.vector.tensor_copy(ylo2[:], pYpair[0:36, 4 * NV:6 * NV])
                    # dims after broadcast: [p,g,a,f,v] -> [p,g,f,a,v]
                    ylo_b = (ylo2[:]
                             .rearrange("p (a v) -> p a v", a=2)
                             [:, None, :, None, :]
                             .broadcast_to([36, 2, 2, 4, NV])
                             .rearrange("p g a f v -> p g f a v"))
                    pa_lo = prodpool.tile([36, PH16], BF16, tag="pa_lo")
                    nc.vector.tensor_mul(
                        pa_lo[:].rearrange("p (g f a v) -> p g f a v",
                                           g=2, f=4, a=2),
                        k_lo2[:].rearrange("p (g f a v) -> p g f a v",
                                           g=2, f=4, a=2),
                        ylo_b)
                    # fxr = Kr*Yr - Ki*Yi ; fxi = Kr*Yi + Ki*Yr.  fxr goes to
                    # fx8 rows {0-3,32-35}; fxi lands in a scratch tile and a
                    # local DMA places it at rows {4-7,36-39} (engines cannot
                    # write at partition offset 4, DMA can), so C' does the lo
                    # contribution with ONE K=8 matmul per phase.
                    fx8 = fxlopool.tile([40, PH4], BF16, tag="fx8")
                    nc.vector.tensor_sub(
                        fx8[0:36].rearrange("p (f v) -> p f v", f=4),
                        pa_lo[:, 0:8 * NV].rearrange("p (f a v) -> p f a v",
                                                     f=4, a=2)[:, :, 0, :],
                        pa_lo[:, 8 * NV:PH16].rearrange("p (f a v) -> p f a v",
                                                        f=4, a=2)[:, :, 1, :])
                    fxi_lo = fxlopool.tile([36, PH4], BF16, tag="fxi_lo")
                    nc.vector.tensor_add(
                        fxi_lo[:].rearrange("p (f v) -> p f v", f=4),
                        pa_lo[:, 0:8 * NV].rearrange("p (f a v) -> p f a v",
                                                     f=4, a=2)[:, :, 1, :],
                        pa_lo[:, 8 * NV:PH16].rearrange("p (f a v) -> p f a v",
                                                        f=4, a=2)[:, :, 0, :])
                    nc.gpsimd.dma_start(fx8[4:8], fxi_lo[0:4])
                    nc.gpsimd.dma_start(fx8[36:40], fxi_lo[32:36])

                    # prefetch next channel's constants during pair 0
                    if pr == 0 and ci + 1 < n_chan:
                        nxt = prep_channel(ci + 1)

                    # previous image's D-stage fills the PE gap
                    if pending:
                        emit_D(pending.pop())

                    for sub in range(2):
                        bi = 2 * pr + sub
                        img = ci * n_batch + bi
                        y_hi = yhis[sub]

                        # ---- hi cmul (DVE, bf16 2x) ----
                        y_hi_b = (y_hi[:]
                                  .rearrange("p (a v) -> p a v", a=2)
                                  [:, None, :, None, :]
                                  .broadcast_to([128, 2, 2, 4, NV])
                                  .rearrange("p g a f v -> p g f a v"))
                        pa_hi = prodpool.tile([128, PH16], BF16, tag="pa_hi")
                        nc.vector.tensor_mul(
                            pa_hi[:].rearrange("p (g f a v) -> p g f a v",
                                               g=2, f=4, a=2),
                            k_hi[:].rearrange("p (g f a v) -> p g f a v",
                                              g=2, f=4, a=2),
                            y_hi_b)
                        fxr_hi = fxpool.tile([128, PH4], BF16, tag="fxr_hi")
                        nc.vector.tensor_sub(
                            fxr_hi[:].rearrange("p (f v) -> p f v", f=4),
                            pa_hi[:, 0:8 * NV].rearrange(
                                "p (f a v) -> p f a v", f=4, a=2)[:, :, 0, :],
                            pa_hi[:, 8 * NV:PH16].rearrange(
                                "p (f a v) -> p f a v", f=4, a=2)[:, :, 1, :])
                        fxi_hi = fxpool.tile([128, PH4], BF16, tag="fxi_hi")
                        nc.vector.tensor_add(
                            fxi_hi[:].rearrange("p (f v) -> p f v", f=4),
                            pa_hi[:, 0:8 * NV].rearrange(
                                "p (f a v) -> p f a v", f=4, a=2)[:, :, 1, :],
                            pa_hi[:, 8 * NV:PH16].rearrange(
                                "p (f a v) -> p f a v", f=4, a=2)[:, :, 0, :])

                        # ---- C': pT1[v, p*256 + (T1r|T1i)] ----
                        pT1 = pt1_pool.tile([NV, 1024], F32, tag="pT1")
                        lo_sl = slice(32 * sub, 32 * sub + 8)
                        for p in range(4):
                            o = pT1[:, ts(p, 256)]
                            nc.tensor.matmul(o, fxr_hi[:, ts(p, NV)],
                                             n12[:, 0:256],
                                             start=True, stop=False)
                            nc.tensor.matmul(o, fxi_hi[:, ts(p, NV)],
                                             n12[:, 256:512],
                                             start=False, stop=False)
                            nc.tensor.matmul(o, fx8[lo_sl, ts(p, NV)],
                                             n12lo[lo_sl, :],
                                             start=False, stop=True)

                        t1sb = t1pool.tile([NV, 1024], BF16, tag="t1sb")
                        nc.scalar.activation(t1sb[:, 0:512], pT1[:, 0:512],
                                             AF.Copy)
                        nc.vector.tensor_copy(t1sb[:, 512:1024],
                                              pT1[:, 512:1024])

                        if pending:
                            emit_D(pending.pop())
                        pending.append((t1sb, img))

            emit_D(pending.pop())

    nc.compile()
    return nc


# --------------------------------------------------------------------------
# public entry point: full inputs in, full output out
# --------------------------------------------------------------------------

def kernel(x, weight, bias):
    global LAST_EXEC_NS
    x = np.ascontiguousarray(np.asarray(x, dtype=np.float32))
    consts = _host_constants(weight, bias)

    nc = build_nc()

    in_maps = []
    for core in range(NCORES):
        c0 = core * CPC
        xs = np.ascontiguousarray(x[:, c0:c0 + CPC].transpose(1, 2, 0, 3))
        in_maps.append({
            "x": xs,
            "k_hi": np.ascontiguousarray(consts["k_hi"][c0:c0 + CPC]),
            "k_lo2": np.ascontiguousarray(consts["k_lo2"][c0:c0 + CPC]),
            "gt": consts["gt"],
            "mab": consts["mab"],
            "n12": consts["n12"],
            "n12lo": consts["n12lo"],
            "rcs": consts["rcs"],
        })

    trace = os.environ.get("KERNEL_TRACE", "0") == "1"
    tmpdir = os.environ.get("KERNEL_TMPDIR") or None
    res = run_bass_kernel_spmd(nc, in_maps, list(range(NCORES)), trace=trace,
                               tmpdir=tmpdir)
    LAST_EXEC_NS = res.exec_time_ns

    out = np.empty((B, C, 2 * N0, 2 * N0), dtype=np.float32)
    for core in range(NCORES):
        c0 = core * CPC
        o = res.results[core]["out"].reshape(CPC, B, 2 * N0, 2 * N0)
        out[:, c0:c0 + CPC] = o.transpose(1, 0, 2, 3)
    return out
